# revision 32
# baseline (speedup 1.0000x reference)
"""JSONTreeLSTM Trainium2 kernel: 8-core data-parallel over K=4096 array children.

Layout: transposed — [128 partitions = mem/gate dims, K_loc=512 free = array index].
The number-embedding + running-stat normalization collapses algebraically into the
gate computation: gates = W_hh @ h + u' (x) x_raw_t + v', with
u' = s_c * (W_ih[:,128:] @ w_num), v' = W_ih[:,128:] @ b_num + b_ih + b_hh - m_c*u'
(s_c, m_c = the post-cap running stats, constant for all flat indices >= 100).
The 100 prefix-normalized elements (flat idx < 100 = numbers[0, :100], core 0 only)
are patched into x via x_eff = x_norm/s_c + m_c so the same affine maps them right.

Pipeline (per core): CH=2 independent k-chains of KH=256, software-pipelined
so each chain's serial cycle (matmul burst -> sigmoid -> cell ops -> tanh ->
h-update) overlaps the other chain's on the shared engines. Per step per chain:
  PE:  2 rank-4 x-inject matmuls ([u_j;v_j;u_j+1;v_j+1] @ 4-row [x;1] block
       pattern), one per psum bank (a bank may only be "opened" once per
       accumulation group - a second start=True on the same bank wipes the
       open region), placed at row-groups 0/32/64/96 so all 4 run
       concurrently; then 4 W_hh matmuls accumulate (start=False).
  ACT: one sigmoid over all 4 gates [128, 1024] psum->bf16 (g pre-scaled x2).
  DVE: cell in half-c form (state c' = c/2, so sigmoid(i)*tanh(g)/2 =
       (sig(2g)-0.5)*sig(i) fuses into one scalar_tensor_tensor):
       m2 = (sg2-0.5)*si [STT]; m1 = sf*c' [TT bf16 2x]; c2' = m1+m2 [TT]
  ACT: tc = Tanh(2*c2')
  DVE: h2 = so*tc [TT]
All state/gate tensors bf16 (rel err ~2e-3, tolerance 2e-2). Engine FIFO
emission order per step: sigA, dveA(c2-chain), sigB, tanhA, dveB(c2-chain),
h2A, tanhB, h2B.

Final child-sums (fc' = sum sigmoid(W_fh h + b_fh)*c' via STT accum_out,
hbar = sum h) are computed per core -> out [128, 2]; the host gathers the 8
cores, sums (fc = 2*sum fc'), and runs the tiny root tree-LSTM in numpy
(part of unsharding; removes the AllReduce).
"""
import sys

sys.path.insert(0, "/opt/trn_rl_repo")
import numpy as np
import concourse.bacc as bacc
import concourse.mybir as mybir
import concourse.tile as tile
from concourse import bass_utils

F32 = mybir.dt.float32
BF16 = mybir.dt.bfloat16
AF = mybir.ActivationFunctionType
OP = mybir.AluOpType
AX = mybir.AxisListType

K, L, MEM, NCORES = 4096, 128, 128, 8
KLOC = K // NCORES  # 512
STATS_CAP = 100
CH = 2
KH = KLOC // CH  # 256

_compiled = {}


def _build(n_cores=NCORES):
    nc = bacc.Bacc("TRN2", target_bir_lowering=False, debug=False,
                   num_devices=n_cores)

    # per t, chain: [4, 512] block pattern
    #   row0 = [x_a | 0], row1 = [1 | 0], row2 = [0 | x_a], row3 = [0 | 1]
    # so one rank-4 matmul against [u_j;v_j;u_j+1;v_j+1] opens a full psum bank
    # covering two gates (avoids two start=True groups per bank, which the HW
    # does not support: a second start wipes the first open region). The four
    # (chain, bank) x-injects go to row-groups 0/32/64/96 so they run
    # concurrently in the PE array.
    xq_d = nc.dram_tensor("xq", [L, CH, 4, 2 * KH], BF16,
                          kind="ExternalInput").ap()
    whhT_d = nc.dram_tensor("whhT", [MEM, 4 * MEM], BF16,
                            kind="ExternalInput").ap()  # W_hh.T, g-block x2
    uvq_d = nc.dram_tensor("uvq", [2, 4, MEM], BF16,
                           kind="ExternalInput").ap()  # [u0;v0;u1;v1],[u2;v2;u3;v3]
    wfhT_d = nc.dram_tensor("wfhT", [MEM, MEM], BF16, kind="ExternalInput").ap()
    bfh_d = nc.dram_tensor("bfh", [MEM, 1], F32, kind="ExternalInput").ap()
    out_d = nc.dram_tensor("out", [MEM, 2], F32, kind="ExternalOutput").ap()

    with tile.TileContext(nc) as tc:
        with tc.tile_pool(name="const", bufs=1) as cp, \
             tc.tile_pool(name="state", bufs=2) as sp, \
             tc.tile_pool(name="xrow", bufs=8) as xp, \
             tc.tile_pool(name="psum", bufs=2, space="PSUM") as pp:

            whhT = cp.tile([MEM, 4 * MEM], BF16, tag="whhT")
            uvq = cp.tile([100, MEM], BF16, tag="uvq")
            wfhT = cp.tile([MEM, MEM], BF16, tag="wfhT")
            bfh = cp.tile([MEM, 1], F32, tag="bfh")
            for t_, d_ in [(whhT, whhT_d), (wfhT, wfhT_d), (bfh, bfh_d)]:
                nc.sync.dma_start(t_[:], d_[:])
            for a in range(CH):
                for b in range(2):
                    q = 32 * (2 * a + b)
                    nc.sync.dma_start(uvq[q:q + 4, :], uvq_d[b])

            h = []
            c = []
            for a in range(CH):
                ht = sp.tile([MEM, KH], BF16, tag=f"h{a}", name=f"h{a}_init")
                ct = sp.tile([MEM, KH], BF16, tag=f"c{a}", name=f"c{a}_init")
                nc.any.memset(ht[:], 0.0)
                nc.any.memset(ct[:], 0.0)
                h.append(ht)
                c.append(ct)

            for t in range(L):
                xq = xp.tile([100, 2 * KH], BF16, tag="xq", name=f"xq_{t}")
                for a in range(CH):
                    for b in range(2):
                        q = 32 * (2 * a + b)
                        eng = nc.sync if b == 0 else nc.gpsimd
                        eng.dma_start(xq[q:q + 4, :], xq_d[t, a])

                # ---- PE: per-chain psum gates = x-inject + W_hh @ h ----
                gp = []
                for a in range(CH):
                    g = pp.tile([MEM, 4 * KH], F32, tag=f"gp{a}",
                                name=f"gp{a}_{t}")
                    gp.append(g)
                    for b in range(2):
                        q = 32 * (2 * a + b)
                        nc.tensor.matmul(g[:, b * 2 * KH:(b + 1) * 2 * KH],
                                         uvq[q:q + 4, :],
                                         xq[q:q + 4, :],
                                         start=True, stop=False,
                                         tile_position=(q, 0))
                for a in range(CH):
                    for j in range(4):
                        nc.tensor.matmul(gp[a][:, j * KH:(j + 1) * KH],
                                         whhT[:, j * MEM:(j + 1) * MEM],
                                         h[a][:], start=False,
                                         stop=(j % 2 == 1))

                # ---- ACT/DVE software pipeline across the two chains ----
                sg = [None] * CH
                c2 = [None] * CH
                so = [None] * CH
                tc_ = [None] * CH

                def sig(a):
                    s = sp.tile([MEM, 4 * KH], BF16, tag=f"sg{a}",
                                name=f"sg{a}_{t}")
                    nc.scalar.activation(s[:], gp[a][:], AF.Sigmoid)
                    sg[a] = s

                def cell(a):
                    # state is c' = c/2:  c2' = (sg2-0.5)*si + sf*c'
                    # ((sg2-0.5)*si = sigmoid(i)*tanh(g)/2); tanh(c)=Tanh(2c')
                    s = sg[a]
                    si = s[:, 0:KH]
                    sf = s[:, KH:2 * KH]
                    sg2 = s[:, 2 * KH:3 * KH]
                    so[a] = s[:, 3 * KH:4 * KH]
                    m2 = sp.tile([MEM, KH], BF16, tag=f"w{a}", name=f"w{a}_{t}")
                    m1 = sp.tile([MEM, KH], BF16, tag=f"m1{a}",
                                 name=f"m1{a}_{t}")
                    cn = sp.tile([MEM, KH], BF16, tag=f"c{a}", name=f"c{a}_{t}")
                    nc.vector.scalar_tensor_tensor(m2, sg2, 0.5, si,
                                                   op0=OP.subtract,
                                                   op1=OP.mult)
                    nc.vector.tensor_mul(m1, sf, c[a][:])
                    nc.vector.tensor_add(cn, m1, m2)
                    c2[a] = cn

                def tanhc(a):
                    tcn = sp.tile([MEM, KH], BF16, tag=f"tc{a}",
                                  name=f"tc{a}_{t}")
                    nc.scalar.activation(tcn[:], c2[a][:], AF.Tanh, scale=2.0)
                    tc_[a] = tcn

                def hout(a):
                    hn = sp.tile([MEM, KH], BF16, tag=f"h{a}", name=f"h{a}_{t}")
                    nc.vector.tensor_mul(hn, so[a], tc_[a])
                    h[a] = hn
                    c[a] = c2[a]

                sig(0)
                cell(0)
                sig(1)
                tanhc(0)
                cell(1)
                hout(0)
                tanhc(1)
                hout(1)

            # ---- per-core child-sum partials ----
            part = cp.tile([MEM, 4], F32, tag="part")
            junk = cp.tile([MEM, KH], BF16, tag="junk")
            for a in range(CH):
                fgp = pp.tile([MEM, KH], F32, tag=f"gp{a}", name=f"fgp{a}")
                nc.tensor.matmul(fgp[:], wfhT[:], h[a][:], start=True,
                                 stop=True)
                fg = sp.tile([MEM, KH], BF16, tag=f"sg{a}", name=f"fg{a}")
                nc.scalar.activation(fg[:], fgp[:], AF.Sigmoid, bias=bfh[:])
                nc.vector.scalar_tensor_tensor(
                    junk[:], fg[:], 1.0, c[a][:], op0=OP.mult, op1=OP.mult,
                    accum_out=part[:, a:a + 1])
                nc.vector.reduce_sum(part[:, 2 + a:3 + a], h[a][:], axis=AX.X)
            outs = cp.tile([MEM, 2], F32, tag="outs")
            nc.vector.tensor_add(outs[:, 0:1], part[:, 0:1], part[:, 1:2])
            nc.vector.tensor_add(outs[:, 1:2], part[:, 2:3], part[:, 3:4])
            nc.sync.dma_start(out_d[:], outs[:])

    nc.compile()
    return nc


def _prep_inputs(numbers, w_num, b_num, W_ih, W_hh, b_ih, b_hh,
                 W_fh, b_fh, W_iouh, b_iouh, W_lout, b_lout):
    f = np.float32
    numbers = np.ascontiguousarray(numbers, f)

    # Running-stat normalization (reference semantics), first STATS_CAP elems.
    x100 = numbers.reshape(-1)[:STATS_CAP].astype(f)
    kk = np.arange(1, STATS_CAP + 1, dtype=f)
    cs = np.cumsum(x100, dtype=f)
    css = np.cumsum(x100 * x100, dtype=f)
    mean_k = cs / kk
    var_k = np.maximum(css / kk - mean_k * mean_k, 0.0)
    std_k = np.sqrt(var_k)
    use_k = (kk > 3.0) & (std_k > 1e-8)
    inv_k = np.where(use_k, 1.0 / np.where(use_k, std_k, 1.0), 1.0).astype(f)
    x_norm0 = (x100 - mean_k) * inv_k
    m_c = float(mean_k[-1])
    s_c = float(inv_k[-1])

    Wr = np.asarray(W_ih, f)[:, MEM:]                      # [512, 128]
    u = (Wr @ np.asarray(w_num, f)) * s_c
    v = (Wr @ np.asarray(b_num, f) + np.asarray(b_ih, f)
         + np.asarray(b_hh, f) - m_c * u)
    whhT = np.asarray(W_hh, f).T.copy()                    # [128, 512]
    whhT[:, 2 * MEM:3 * MEM] *= 2.0                        # g-gate -> 2g
    u = u.astype(f).copy()
    v = v.astype(f).copy()
    u[2 * MEM:3 * MEM] *= 2.0
    v[2 * MEM:3 * MEM] *= 2.0

    try:
        import ml_dtypes
        bf16 = ml_dtypes.bfloat16
    except ImportError:
        import jax.numpy as jnp
        bf16 = jnp.bfloat16
    uq = u.reshape(4, MEM)
    vq = v.reshape(4, MEM)
    uvq = np.zeros((2, 4, MEM), f)
    uvq[0] = [uq[0], vq[0], uq[1], vq[1]]
    uvq[1] = [uq[2], vq[2], uq[3], vq[3]]
    shared = {
        "whhT": whhT.astype(bf16),
        "uvq": uvq.astype(bf16),
        "wfhT": np.asarray(W_fh, f).T.copy().astype(bf16),
        "bfh": np.asarray(b_fh, f).reshape(MEM, 1).copy(),
    }
    in_maps = []
    for cid in range(NCORES):
        m = dict(shared)
        xT = np.ascontiguousarray(numbers[cid * KLOC:(cid + 1) * KLOC, :].T, f)
        if cid == 0:
            # patch flat elements < 100 (k=0 column -> x row entries [t, 0])
            # so the constant affine reproduces their prefix normalization
            x_eff = x_norm0 / s_c + m_c
            xT[:STATS_CAP, 0] = x_eff
        # block pattern per chain a: row0=[x_a|0], row1=[1|0], row2=[0|x_a],
        # row3=[0|1]  (2*KH = 512 wide = one psum bank covering 2 gates)
        xqm = np.zeros((L, CH, 4, 2 * KH), f)
        for a in range(CH):
            xa = xT[:, a * KH:(a + 1) * KH]               # [L, KH]
            xqm[:, a, 0, 0:KH] = xa
            xqm[:, a, 1, 0:KH] = 1.0
            xqm[:, a, 2, KH:2 * KH] = xa
            xqm[:, a, 3, KH:2 * KH] = 1.0
        m["xq"] = xqm.astype(bf16)
        in_maps.append(m)
    return in_maps


def _sigmoid(x):
    return 1.0 / (1.0 + np.exp(-x))


def kernel(**inputs):
    if "nc" not in _compiled:
        _compiled["nc"] = _build()
    nc = _compiled["nc"]
    in_maps = _prep_inputs(**inputs)
    last_err = None
    for _attempt in range(3):
        try:
            res = bass_utils.run_bass_kernel_spmd(nc, in_maps,
                                                  core_ids=list(range(NCORES)))
            break
        except Exception as e:  # transient NRT device faults happen rarely
            last_err = e
    else:
        raise last_err

    f = np.float32
    fc_sum = np.zeros(MEM, f)
    hbar = np.zeros(MEM, f)
    for cid in range(NCORES):
        o = np.asarray(res.results[cid]["out"], f)          # [128, 2]
        fc_sum += 2.0 * o[:, 0]    # device accumulates fg*c' with c' = c/2
        hbar += o[:, 1]

    # root tree-LSTM (tiny) on host — part of unsharding the 8 partials
    W_iouh = np.asarray(inputs["W_iouh"], f)
    b_iouh = np.asarray(inputs["b_iouh"], f)
    W_lout = np.asarray(inputs["W_lout"], f)
    b_lout = np.asarray(inputs["b_lout"], f)
    iou = W_iouh @ hbar + b_iouh
    i, o, u = iou[:MEM], iou[MEM:2 * MEM], iou[2 * MEM:]
    c_root = _sigmoid(i) * np.tanh(u) + fc_sum
    h_root = _sigmoid(o) * np.tanh(c_root)
    h_hat = W_lout @ h_root + b_lout
    return np.concatenate([c_root, h_hat])[None, :].astype(np.float32)


# revision 33
# speedup vs baseline: 1.0003x; 1.0003x over previous
"""JSONTreeLSTM Trainium2 kernel: 8-core data-parallel over K=4096 array children.

Layout: transposed — [128 partitions = mem/gate dims, K_loc=512 free = array index].
The number-embedding + running-stat normalization collapses algebraically into the
gate computation: gates = W_hh @ h + u' (x) x_raw_t + v', with
u' = s_c * (W_ih[:,128:] @ w_num), v' = W_ih[:,128:] @ b_num + b_ih + b_hh - m_c*u'
(s_c, m_c = the post-cap running stats, constant for all flat indices >= 100).
The 100 prefix-normalized elements (flat idx < 100 = numbers[0, :100], core 0 only)
are patched into x via x_eff = x_norm/s_c + m_c so the same affine maps them right.

Pipeline (per core): CH=2 independent k-chains of KH=256, software-pipelined
so each chain's serial cycle (matmul burst -> sigmoid -> cell ops -> tanh ->
h-update) overlaps the other chain's on the shared engines. Per step per chain:
  PE:  2 rank-4 x-inject matmuls ([u_j;v_j;u_j+1;v_j+1] @ 4-row [x;1] block
       pattern), one per psum bank (a bank may only be "opened" once per
       accumulation group - a second start=True on the same bank wipes the
       open region), placed at row-groups 0/32/64/96 so all 4 run
       concurrently; then 4 W_hh matmuls accumulate (start=False).
  ACT: one sigmoid over all 4 gates [128, 1024] psum->bf16 (g pre-scaled x2).
  DVE: cell in half-c form (state c' = c/2, so sigmoid(i)*tanh(g)/2 =
       (sig(2g)-0.5)*sig(i) fuses into one scalar_tensor_tensor):
       m2 = (sg2-0.5)*si [STT]; m1 = sf*c' [TT bf16 2x]; c2' = m1+m2 [TT]
  ACT: tc = Tanh(2*c2')
  DVE: h2 = so*tc [TT]
All state/gate tensors bf16 (rel err ~2e-3, tolerance 2e-2). Engine FIFO
emission order per step: sigA, dveA(c2-chain), sigB, tanhA, dveB(c2-chain),
h2A, tanhB, h2B.

Final child-sums (fc' = sum sigmoid(W_fh h + b_fh)*c' via STT accum_out,
hbar = sum h) are computed per core -> out [128, 2]; the host gathers the 8
cores, sums (fc = 2*sum fc'), and runs the tiny root tree-LSTM in numpy
(part of unsharding; removes the AllReduce).
"""
import sys

sys.path.insert(0, "/opt/trn_rl_repo")
import numpy as np
import concourse.bacc as bacc
import concourse.mybir as mybir
import concourse.tile as tile
from concourse import bass_utils

F32 = mybir.dt.float32
BF16 = mybir.dt.bfloat16
AF = mybir.ActivationFunctionType
OP = mybir.AluOpType
AX = mybir.AxisListType

K, L, MEM, NCORES = 4096, 128, 128, 8
KLOC = K // NCORES  # 512
STATS_CAP = 100
CH = 2
KH = KLOC // CH  # 256

_compiled = {}


def _build(n_cores=NCORES):
    nc = bacc.Bacc("TRN2", target_bir_lowering=False, debug=False,
                   num_devices=n_cores)

    # per t, chain: [4, 512] block pattern
    #   row0 = [x_a | 0], row1 = [1 | 0], row2 = [0 | x_a], row3 = [0 | 1]
    # so one rank-4 matmul against [u_j;v_j;u_j+1;v_j+1] opens a full psum bank
    # covering two gates (avoids two start=True groups per bank, which the HW
    # does not support: a second start wipes the first open region). The four
    # (chain, bank) x-injects go to row-groups 0/32/64/96 so they run
    # concurrently in the PE array.
    xq_d = nc.dram_tensor("xq", [L, CH, 4, 2 * KH], BF16,
                          kind="ExternalInput").ap()
    whhT_d = nc.dram_tensor("whhT", [MEM, 4 * MEM], BF16,
                            kind="ExternalInput").ap()  # W_hh.T, g-block x2
    uvq_d = nc.dram_tensor("uvq", [2, 4, MEM], BF16,
                           kind="ExternalInput").ap()  # [u0;v0;u1;v1],[u2;v2;u3;v3]
    wfhT_d = nc.dram_tensor("wfhT", [MEM, MEM], BF16, kind="ExternalInput").ap()
    bfh_d = nc.dram_tensor("bfh", [MEM, 1], F32, kind="ExternalInput").ap()
    out_d = nc.dram_tensor("out", [MEM, 2], F32, kind="ExternalOutput").ap()

    with tile.TileContext(nc) as tc:
        with tc.tile_pool(name="const", bufs=1) as cp, \
             tc.tile_pool(name="state", bufs=3) as sp, \
             tc.tile_pool(name="xrow", bufs=8) as xp, \
             tc.tile_pool(name="psum", bufs=2, space="PSUM") as pp:

            whhT = cp.tile([MEM, 4 * MEM], BF16, tag="whhT")
            uvq = cp.tile([100, MEM], BF16, tag="uvq")
            wfhT = cp.tile([MEM, MEM], BF16, tag="wfhT")
            bfh = cp.tile([MEM, 1], F32, tag="bfh")
            for t_, d_ in [(whhT, whhT_d), (wfhT, wfhT_d), (bfh, bfh_d)]:
                nc.sync.dma_start(t_[:], d_[:])
            for a in range(CH):
                for b in range(2):
                    q = 32 * (2 * a + b)
                    nc.sync.dma_start(uvq[q:q + 4, :], uvq_d[b])

            h = []
            c = []
            for a in range(CH):
                ht = sp.tile([MEM, KH], BF16, tag=f"h{a}", name=f"h{a}_init")
                ct = sp.tile([MEM, KH], BF16, tag=f"c{a}", name=f"c{a}_init")
                nc.any.memset(ht[:], 0.0)
                nc.any.memset(ct[:], 0.0)
                h.append(ht)
                c.append(ct)

            for t in range(L):
                xq = xp.tile([100, 2 * KH], BF16, tag="xq", name=f"xq_{t}")
                for a in range(CH):
                    for b in range(2):
                        q = 32 * (2 * a + b)
                        eng = nc.sync if b == 0 else nc.gpsimd
                        eng.dma_start(xq[q:q + 4, :], xq_d[t, a])

                # ---- PE: per-chain psum gates = x-inject + W_hh @ h ----
                gp = []
                for a in range(CH):
                    g = pp.tile([MEM, 4 * KH], F32, tag=f"gp{a}",
                                name=f"gp{a}_{t}")
                    gp.append(g)
                    for b in range(2):
                        q = 32 * (2 * a + b)
                        nc.tensor.matmul(g[:, b * 2 * KH:(b + 1) * 2 * KH],
                                         uvq[q:q + 4, :],
                                         xq[q:q + 4, :],
                                         start=True, stop=False,
                                         tile_position=(q, 0))
                for a in range(CH):
                    for j in range(4):
                        nc.tensor.matmul(gp[a][:, j * KH:(j + 1) * KH],
                                         whhT[:, j * MEM:(j + 1) * MEM],
                                         h[a][:], start=False,
                                         stop=(j % 2 == 1))

                # ---- ACT/DVE software pipeline across the two chains ----
                sg = [None] * CH
                c2 = [None] * CH
                so = [None] * CH
                tc_ = [None] * CH

                def sig(a):
                    s = sp.tile([MEM, 4 * KH], BF16, tag=f"sg{a}",
                                name=f"sg{a}_{t}")
                    nc.scalar.activation(s[:], gp[a][:], AF.Sigmoid)
                    sg[a] = s

                def cell(a):
                    # state is c' = c/2:  c2' = (sg2-0.5)*si + sf*c'
                    # ((sg2-0.5)*si = sigmoid(i)*tanh(g)/2); tanh(c)=Tanh(2c')
                    s = sg[a]
                    si = s[:, 0:KH]
                    sf = s[:, KH:2 * KH]
                    sg2 = s[:, 2 * KH:3 * KH]
                    so[a] = s[:, 3 * KH:4 * KH]
                    m2 = sp.tile([MEM, KH], BF16, tag=f"w{a}", name=f"w{a}_{t}")
                    m1 = sp.tile([MEM, KH], BF16, tag=f"m1{a}",
                                 name=f"m1{a}_{t}")
                    cn = sp.tile([MEM, KH], BF16, tag=f"c{a}", name=f"c{a}_{t}")
                    nc.vector.scalar_tensor_tensor(m2, sg2, 0.5, si,
                                                   op0=OP.subtract,
                                                   op1=OP.mult)
                    nc.vector.tensor_mul(m1, sf, c[a][:])
                    nc.vector.tensor_add(cn, m1, m2)
                    c2[a] = cn

                def tanhc(a):
                    tcn = sp.tile([MEM, KH], BF16, tag=f"tc{a}",
                                  name=f"tc{a}_{t}")
                    nc.scalar.activation(tcn[:], c2[a][:], AF.Tanh, scale=2.0)
                    tc_[a] = tcn

                def hout(a):
                    hn = sp.tile([MEM, KH], BF16, tag=f"h{a}", name=f"h{a}_{t}")
                    nc.vector.tensor_mul(hn, so[a], tc_[a])
                    h[a] = hn
                    c[a] = c2[a]

                sig(0)
                cell(0)
                sig(1)
                tanhc(0)
                cell(1)
                hout(0)
                tanhc(1)
                hout(1)

            # ---- per-core child-sum partials ----
            part = cp.tile([MEM, 4], F32, tag="part")
            junk = cp.tile([MEM, KH], BF16, tag="junk")
            for a in range(CH):
                fgp = pp.tile([MEM, KH], F32, tag=f"gp{a}", name=f"fgp{a}")
                nc.tensor.matmul(fgp[:], wfhT[:], h[a][:], start=True,
                                 stop=True)
                fg = sp.tile([MEM, KH], BF16, tag=f"sg{a}", name=f"fg{a}")
                nc.scalar.activation(fg[:], fgp[:], AF.Sigmoid, bias=bfh[:])
                nc.vector.scalar_tensor_tensor(
                    junk[:], fg[:], 1.0, c[a][:], op0=OP.mult, op1=OP.mult,
                    accum_out=part[:, a:a + 1])
                nc.vector.reduce_sum(part[:, 2 + a:3 + a], h[a][:], axis=AX.X)
            outs = cp.tile([MEM, 2], F32, tag="outs")
            nc.vector.tensor_add(outs[:, 0:1], part[:, 0:1], part[:, 1:2])
            nc.vector.tensor_add(outs[:, 1:2], part[:, 2:3], part[:, 3:4])
            nc.sync.dma_start(out_d[:], outs[:])

    nc.compile()
    return nc


def _prep_inputs(numbers, w_num, b_num, W_ih, W_hh, b_ih, b_hh,
                 W_fh, b_fh, W_iouh, b_iouh, W_lout, b_lout):
    f = np.float32
    numbers = np.ascontiguousarray(numbers, f)

    # Running-stat normalization (reference semantics), first STATS_CAP elems.
    x100 = numbers.reshape(-1)[:STATS_CAP].astype(f)
    kk = np.arange(1, STATS_CAP + 1, dtype=f)
    cs = np.cumsum(x100, dtype=f)
    css = np.cumsum(x100 * x100, dtype=f)
    mean_k = cs / kk
    var_k = np.maximum(css / kk - mean_k * mean_k, 0.0)
    std_k = np.sqrt(var_k)
    use_k = (kk > 3.0) & (std_k > 1e-8)
    inv_k = np.where(use_k, 1.0 / np.where(use_k, std_k, 1.0), 1.0).astype(f)
    x_norm0 = (x100 - mean_k) * inv_k
    m_c = float(mean_k[-1])
    s_c = float(inv_k[-1])

    Wr = np.asarray(W_ih, f)[:, MEM:]                      # [512, 128]
    u = (Wr @ np.asarray(w_num, f)) * s_c
    v = (Wr @ np.asarray(b_num, f) + np.asarray(b_ih, f)
         + np.asarray(b_hh, f) - m_c * u)
    whhT = np.asarray(W_hh, f).T.copy()                    # [128, 512]
    whhT[:, 2 * MEM:3 * MEM] *= 2.0                        # g-gate -> 2g
    u = u.astype(f).copy()
    v = v.astype(f).copy()
    u[2 * MEM:3 * MEM] *= 2.0
    v[2 * MEM:3 * MEM] *= 2.0

    try:
        import ml_dtypes
        bf16 = ml_dtypes.bfloat16
    except ImportError:
        import jax.numpy as jnp
        bf16 = jnp.bfloat16
    uq = u.reshape(4, MEM)
    vq = v.reshape(4, MEM)
    uvq = np.zeros((2, 4, MEM), f)
    uvq[0] = [uq[0], vq[0], uq[1], vq[1]]
    uvq[1] = [uq[2], vq[2], uq[3], vq[3]]
    shared = {
        "whhT": whhT.astype(bf16),
        "uvq": uvq.astype(bf16),
        "wfhT": np.asarray(W_fh, f).T.copy().astype(bf16),
        "bfh": np.asarray(b_fh, f).reshape(MEM, 1).copy(),
    }
    in_maps = []
    for cid in range(NCORES):
        m = dict(shared)
        xT = np.ascontiguousarray(numbers[cid * KLOC:(cid + 1) * KLOC, :].T, f)
        if cid == 0:
            # patch flat elements < 100 (k=0 column -> x row entries [t, 0])
            # so the constant affine reproduces their prefix normalization
            x_eff = x_norm0 / s_c + m_c
            xT[:STATS_CAP, 0] = x_eff
        # block pattern per chain a: row0=[x_a|0], row1=[1|0], row2=[0|x_a],
        # row3=[0|1]  (2*KH = 512 wide = one psum bank covering 2 gates)
        xqm = np.zeros((L, CH, 4, 2 * KH), f)
        for a in range(CH):
            xa = xT[:, a * KH:(a + 1) * KH]               # [L, KH]
            xqm[:, a, 0, 0:KH] = xa
            xqm[:, a, 1, 0:KH] = 1.0
            xqm[:, a, 2, KH:2 * KH] = xa
            xqm[:, a, 3, KH:2 * KH] = 1.0
        m["xq"] = xqm.astype(bf16)
        in_maps.append(m)
    return in_maps


def _sigmoid(x):
    return 1.0 / (1.0 + np.exp(-x))


def kernel(**inputs):
    if "nc" not in _compiled:
        _compiled["nc"] = _build()
    nc = _compiled["nc"]
    in_maps = _prep_inputs(**inputs)
    last_err = None
    for _attempt in range(3):
        try:
            res = bass_utils.run_bass_kernel_spmd(nc, in_maps,
                                                  core_ids=list(range(NCORES)))
            break
        except Exception as e:  # transient NRT device faults happen rarely
            last_err = e
    else:
        raise last_err

    f = np.float32
    fc_sum = np.zeros(MEM, f)
    hbar = np.zeros(MEM, f)
    for cid in range(NCORES):
        o = np.asarray(res.results[cid]["out"], f)          # [128, 2]
        fc_sum += 2.0 * o[:, 0]    # device accumulates fg*c' with c' = c/2
        hbar += o[:, 1]

    # root tree-LSTM (tiny) on host — part of unsharding the 8 partials
    W_iouh = np.asarray(inputs["W_iouh"], f)
    b_iouh = np.asarray(inputs["b_iouh"], f)
    W_lout = np.asarray(inputs["W_lout"], f)
    b_lout = np.asarray(inputs["b_lout"], f)
    iou = W_iouh @ hbar + b_iouh
    i, o, u = iou[:MEM], iou[MEM:2 * MEM], iou[2 * MEM:]
    c_root = _sigmoid(i) * np.tanh(u) + fc_sum
    h_root = _sigmoid(o) * np.tanh(c_root)
    h_hat = W_lout @ h_root + b_lout
    return np.concatenate([c_root, h_hat])[None, :].astype(np.float32)
